# revision 17
# baseline (speedup 1.0000x reference)
"""CrossAttention TRN2 kernel.

Full-input contract: kernel(**inputs) takes the unsharded numpy inputs of
  reference.py (q,k,v [2,2048,1024] fp32; Wq/Wk/Wv/Wo [1024,1024]; biases)
and returns the full [2,2048,1024] fp32 output.

Sharding: 8 cores = 2 batch groups x 4 head groups (tensor parallel over
heads).  Core c handles batch c//4 and heads [4*(c%4), 4*(c%4)+4).
Each core computes its heads' Q/K/V projections, attention, and a partial
output projection (row-slice of Wo); the host sums the 4 partials per batch
(no on-device collectives needed).

Per-core dataflow (all matmuls bf16 with fp32 PSUM accumulation):
  - host pre-transposes/casts activations (q^T,k^T,v^T [cin, tok] bf16) and
    weight slices, so contraction dims land on SBUF partitions directly.
  - scores are computed transposed ([ts, tq]) so the PV matmul can contract
    ts on partitions; a ones-column appended to vh yields the softmax
    denominator as PV row 64 for free.
  - exp runs on ScalarE (scale 1/sqrt(d) folded in), FD=1024 per activation.
"""

import numpy as np
import ml_dtypes

BF16 = ml_dtypes.bfloat16

B, TOKENS, C = 2, 2048, 1024
NHEAD, D = 16, 64
NCORES = 8
NGROUP = 4                # head groups (cores per batch)
COUT = C // NGROUP        # 256 head-channels per core
NH = NHEAD // NGROUP      # 4 heads per core

P = 128                   # SBUF partitions


def build_nc(tok=TOKENS, cin=C, cout=COUT, nh=NH):
    """Emit the per-core Bass module. Parametric so a small version can be
    validated in CoreSim quickly. d=64 fixed; cout = nh*64."""
    import concourse.bacc as bacc
    import concourse.tile as tile
    import concourse.mybir as mybir

    d = D
    assert cout == nh * d
    ncin = cin // P               # cin tiles (contraction)
    nt = tok // P                 # token tiles
    nm = max(1, cout // P)        # 128-wide cout chunks (qhT/khT)
    heads_per_chunk = P // d      # 2
    tqb = min(1024, tok)          # tq block (exp FD)
    ntqb = tok // tqb
    sck = min(512, tok)           # matmul moving chunk
    csk = tqb // sck              # chunks per tq block
    nob = max(1, min(2, cin // 512))  # out-proj cout chunks of 512
    ob = cin // nob               # out-proj N per chunk
    nko = cout // P if cout >= P else 1  # out-proj contraction tiles

    fp32 = mybir.dt.float32
    bf16 = mybir.dt.bfloat16

    nc = bacc.Bacc("TRN2", target_bir_lowering=False, debug=False)

    qT = nc.dram_tensor("qT", [cin, tok], bf16, kind="ExternalInput")
    kT = nc.dram_tensor("kT", [cin, tok], bf16, kind="ExternalInput")
    vT = nc.dram_tensor("vT", [cin, tok], bf16, kind="ExternalInput")
    wqT = nc.dram_tensor("wqT", [cin, cout], bf16, kind="ExternalInput")
    wkT = nc.dram_tensor("wkT", [cin, cout], bf16, kind="ExternalInput")
    wvT = nc.dram_tensor("wvT", [cin, cout], bf16, kind="ExternalInput")
    woT = nc.dram_tensor("woT", [cout, cin], bf16, kind="ExternalInput")
    bqv = nc.dram_tensor("bqv", [P, nm], fp32, kind="ExternalInput")
    bkv = nc.dram_tensor("bkv", [P, nm], fp32, kind="ExternalInput")
    bvv = nc.dram_tensor("bvv", [1, cout], fp32, kind="ExternalInput")
    outp = nc.dram_tensor("outp", [tok, cin], fp32, kind="ExternalOutput")

    with tile.TileContext(nc) as tc:
        from contextlib import ExitStack
        with ExitStack() as ctx:
            consts = ctx.enter_context(tc.tile_pool(name="consts", bufs=1))
            xstream = ctx.enter_context(tc.tile_pool(name="xstream", bufs=2))
            vstream = ctx.enter_context(tc.tile_pool(name="vstream", bufs=2))
            persist = ctx.enter_context(tc.tile_pool(name="persist", bufs=1))
            expool = ctx.enter_context(tc.tile_pool(name="expool", bufs=3))
            smalls = ctx.enter_context(tc.tile_pool(name="smalls", bufs=4))
            ostage = ctx.enter_context(tc.tile_pool(name="ostage", bufs=3))
            dscr = ctx.enter_context(
                tc.tile_pool(name="dscr", bufs=2, space="DRAM"))
            psum = ctx.enter_context(
                tc.tile_pool(name="psum", bufs=1, space="PSUM"))

            # ---- constants -------------------------------------------------
            wq_sb = consts.tile([P, ncin, cout], bf16, tag="wq")
            wk_sb = consts.tile([P, ncin, cout], bf16, tag="wk")
            wv_sb = consts.tile([P, ncin, cout], bf16, tag="wv")
            for w_sb, w_h in ((wq_sb, wqT), (wk_sb, wkT), (wv_sb, wvT)):
                nc.sync.dma_start(
                    out=w_sb,
                    in_=w_h[:, :].rearrange("(nb p) co -> p nb co", p=P))
            wo_sb = consts.tile([P, nko, cin], bf16, tag="wo")
            nc.sync.dma_start(
                out=wo_sb,
                in_=woT[:, :].rearrange("(nb p) co -> p nb co", p=P))
            bq_sb = consts.tile([P, nm], fp32, tag="bq")
            bk_sb = consts.tile([P, nm], fp32, tag="bk")
            nc.sync.dma_start(out=bq_sb, in_=bqv[:, :])
            nc.sync.dma_start(out=bk_sb, in_=bkv[:, :])
            bv_sb = consts.tile([P, nh, d], fp32, tag="bv")
            nc.gpsimd.dma_start(
                out=bv_sb,
                in_=bvv[:, :].rearrange("o (h e) -> o h e", h=nh)
                .to_broadcast([P, nh, d]))

            # ---- projections ----------------------------------------------
            # Emission order matters for overlap: K/Q chunk m=0 first (lets
            # head-0 attention + ScalarE exp start early), V projection next
            # (vh[i] consumed by the first PV sweep), remaining chunks after.
            vh_all = persist.tile([P, nt, nh, d + 1], bf16, tag="vh")
            nc.vector.memset(vh_all[:, :, :, d:d + 1], 1.0)
            qh_sb = persist.tile([P, nm, tok], bf16, tag="qh")
            kh_sb = persist.tile([P, nm, tok], bf16, tag="kh")

            def emit_qk_chunk(x_h, w_sb, b_sb, xh_sb, it, m, xtag):
                xt = xstream.tile([P, ncin, sck], bf16, tag=xtag,
                                  name=f"xt_{xtag}_{it}_{m}")
                nc.sync.dma_start(
                    out=xt,
                    in_=x_h[:, :].rearrange("(nb p) t -> p nb t", p=P)
                    [:, :, it * sck:(it + 1) * sck])
                ps = psum.tile([P, sck], fp32, tag="pp", bufs=2, name="psqk")
                for ci in range(ncin):
                    nc.tensor.matmul(
                        ps, w_sb[:, ci, m * P:(m + 1) * P], xt[:, ci, :],
                        start=(ci == 0), stop=(ci == ncin - 1))
                nc.vector.tensor_scalar(
                    out=xh_sb[:, m, it * sck:(it + 1) * sck],
                    in0=ps, scalar1=b_sb[:, m:m + 1], scalar2=None,
                    op0=mybir.AluOpType.add)

            def emit_v_tile(it):
                vt = vstream.tile([P, ncin, P], bf16, tag="vt",
                                  name=f"vt_{it}")
                nc.sync.dma_start(
                    out=vt,
                    in_=vT[:, :].rearrange("(nb p) t -> p nb t", p=P)
                    [:, :, it * P:(it + 1) * P])
                ps = psum.tile([P, cout], fp32, tag="pp", bufs=2, name="psv")
                for ci in range(ncin):
                    nc.tensor.matmul(ps, vt[:, ci, :], wv_sb[:, ci, :],
                                     start=(ci == 0), stop=(ci == ncin - 1))
                nc.vector.tensor_tensor(
                    out=vh_all[:, it, :, 0:d],
                    in0=ps.rearrange("p (h e) -> p h e", h=nh),
                    in1=bv_sb,
                    op=mybir.AluOpType.add)

            for it in range(tok // sck):
                emit_qk_chunk(kT, wk_sb, bk_sb, kh_sb, it, 0, "xk")
                emit_qk_chunk(qT, wq_sb, bq_sb, qh_sb, it, 0, "xq")
            for it in range(nt):
                emit_v_tile(it)
            for m in range(1, nm):
                for it in range(tok // sck):
                    emit_qk_chunk(kT, wk_sb, bk_sb, kh_sb, it, m, "xk")
                    emit_qk_chunk(qT, wq_sb, bq_sb, qh_sb, it, m, "xq")

            # ---- attention per head ---------------------------------------
            att_pair = [persist.tile([P, tok], bf16, tag=f"att{k}",
                                     name=f"att{k}")
                        for k in range(nko)]
            for tb in range(ntqb):
                for h in range(nh):
                    m = h // heads_per_chunk
                    half = h % heads_per_chunk
                    p0 = half * d
                    kh_h = kh_sb[p0:p0 + d, m, :]
                    qh_h = qh_sb[p0:p0 + d, m, :]
                    pv = psum.tile([d + 1, tqb], fp32, tag="pv", bufs=1,
                                   name=f"pv_h{h}_{tb}")
                    for i in range(nt):
                        s_ps = psum.tile([P, tqb], fp32, tag="s", bufs=2,
                                         name="s_ps")
                        for cc in range(csk):
                            q0 = tb * tqb + cc * sck
                            nc.tensor.matmul(
                                s_ps[:, cc * sck:(cc + 1) * sck],
                                kh_h[:, i * P:(i + 1) * P],
                                qh_h[:, q0:q0 + sck],
                                start=True, stop=True)
                        ex = expool.tile([P, tqb], bf16, tag="ex")
                        nc.scalar.activation(
                            out=ex, in_=s_ps,
                            func=mybir.ActivationFunctionType.Exp,
                            scale=float(d) ** -0.5)
                        for cc in range(csk):
                            nc.tensor.matmul(
                                pv[:, cc * sck:(cc + 1) * sck],
                                vh_all[:, i, h, :],
                                ex[:, cc * sck:(cc + 1) * sck],
                                start=(i == 0), stop=(i == nt - 1))
                    # Stage PV (incl. denom row) to SBUF fp32 in one copy so
                    # the PSUM accumulator frees immediately; the reciprocal/
                    # broadcast/normalize chain then runs off critical path.
                    stg = smalls.tile([d + 1, tqb], fp32, tag="stg",
                                      name=f"stg_h{h}_{tb}")
                    nc.vector.tensor_copy(out=stg, in_=pv)
                    rec = smalls.tile([1, tqb], fp32, tag="rec")
                    nc.vector.reciprocal_approx_fast(out=rec,
                                                     in_=stg[d:d + 1, :])
                    rdr = dscr.tile([1, tqb], fp32, tag="rdr")
                    nc.sync.dma_start(out=rdr, in_=rec)
                    rep = smalls.tile([d, tqb], fp32, tag="rep")
                    nc.gpsimd.dma_start(out=rep,
                                        in_=rdr.to_broadcast([d, tqb]))
                    nc.vector.tensor_tensor(
                        out=att_pair[m][p0:p0 + d,
                                        tb * tqb:(tb + 1) * tqb],
                        in0=stg[0:d, :], in1=rep,
                        op=mybir.AluOpType.mult)

                # ---- partial output projection for this token block -------
                for tt in range(tb * (tqb // P), (tb + 1) * (tqb // P)):
                    for n in range(nob):
                        ps = psum.tile([P, ob], fp32, tag="pp", bufs=2,
                                       name="pso")
                        for ko in range(nko):
                            nc.tensor.matmul(
                                ps, att_pair[ko][:, tt * P:(tt + 1) * P],
                                wo_sb[:, ko, n * ob:(n + 1) * ob],
                                start=(ko == 0), stop=(ko == nko - 1))
                        o_sb = ostage.tile([P, ob], fp32, tag="ost")
                        nc.vector.tensor_copy(out=o_sb, in_=ps)
                        nc.sync.dma_start(
                            out=outp[tt * P:(tt + 1) * P,
                                     n * ob:(n + 1) * ob],
                            in_=o_sb)

    nc.compile()
    return nc


def _host_inputs(q, k, v, Wq, Wk, Wv, Wo, bq, bk, bv,
                 tok=TOKENS, cin=C, cout=COUT, ngroup=NGROUP, ncores=NCORES):
    """Build per-core in_maps (host-side shard + transpose + bf16 cast)."""
    nm = max(1, cout // P)
    xT = {}
    for b in range(q.shape[0]):
        xT[('q', b)] = np.ascontiguousarray(q[b].T).astype(BF16)
        xT[('k', b)] = np.ascontiguousarray(k[b].T).astype(BF16)
        xT[('v', b)] = np.ascontiguousarray(v[b].T).astype(BF16)
    in_maps = []
    for core in range(ncores):
        b, g = core // ngroup, core % ngroup
        sl = slice(g * cout, (g + 1) * cout)
        in_maps.append({
            "qT": xT[('q', b)],
            "kT": xT[('k', b)],
            "vT": xT[('v', b)],
            "wqT": np.ascontiguousarray(Wq[sl, :].T).astype(BF16),
            "wkT": np.ascontiguousarray(Wk[sl, :].T).astype(BF16),
            "wvT": np.ascontiguousarray(Wv[sl, :].T).astype(BF16),
            "woT": np.ascontiguousarray(Wo[:, sl].T).astype(BF16),
            "bqv": np.ascontiguousarray(
                bq[sl].reshape(nm, P).T).astype(np.float32),
            "bkv": np.ascontiguousarray(
                bk[sl].reshape(nm, P).T).astype(np.float32),
            "bvv": np.ascontiguousarray(bv[sl][None, :]).astype(np.float32),
        })
    return in_maps


_NC_CACHE = {}


def _get_nc():
    if "nc" not in _NC_CACHE:
        _NC_CACHE["nc"] = build_nc()
    return _NC_CACHE["nc"]


def kernel(q, k, v, Wq, bq, Wk, bk, Wv, bv, Wo, bo):
    from concourse.bass_utils import run_bass_kernel_spmd

    q = np.asarray(q, dtype=np.float32)
    k = np.asarray(k, dtype=np.float32)
    v = np.asarray(v, dtype=np.float32)
    nc = _get_nc()
    in_maps = _host_inputs(q, k, v,
                           np.asarray(Wq, np.float32), np.asarray(Wk, np.float32),
                           np.asarray(Wv, np.float32), np.asarray(Wo, np.float32),
                           np.asarray(bq, np.float32), np.asarray(bk, np.float32),
                           np.asarray(bv, np.float32))
    res = run_bass_kernel_spmd(nc, in_maps, core_ids=list(range(NCORES)))
    parts = [r["outp"] for r in res.results]
    out = np.stack(
        [sum(parts[b * NGROUP:(b + 1) * NGROUP]) for b in range(B)], axis=0)
    out = out + np.asarray(bo, np.float32)[None, None, :]
    return out.astype(np.float32)
